# revision 19
# baseline (speedup 1.0000x reference)
"""Trainium2 Bass kernel for nn_AttnLayer (additive-attention pooling layer).

Reference computation (per batch b):
    e = e_hiddens @ We_w.T + We_b            # [S, F]
    d = Wd_w @ d_hiddens[b]                  # [F]
    h = tanh(d + e)                          # [S, F]
    s = h @ v_w[0] + v_b                     # [S]
    a = softmax(s)                           # [S]
    out[b] = a @ e_hiddens[b]                # [D]

Strategy (8 cores, data-parallel over batch B=32 -> 4 per core):
  x pre-transposed ON HOST to [d-partition, s-free] bf16, packed in PAIRS of
  1024-long s-chunks (one DMA + one exp + one big DVE mult per pair).
  Per pair (2 chunks of the same batch):
    PE : e^T[f,s] = sum_k wet^T @ xt  (bf16, 8 LDW + 16 MM per chunk);
         scores replicated to 128 partitions via v-replicated stationary
    ACT: h = tanh(e^T + d_b)  ;  a2 = exp(sc2 + v_b) over BOTH chunks in one
         op (accum_out -> pair partial of softmax Z)
    weighted sum x*a, split by engine rates:
      DVE: ONE broadcast tensor_tensor (2x bf16 mode) multiplies k-slices
           0..6 of both chunks; slices 0..4 are then reduced by an IN-PLACE
           bf16 halving fold tree (every fold also 2x) whose 32-wide tails
           drop into a per-batch partial tile
      Pool: k=7 mults (kept small: Pool shares an SBUF port with DVE)
      ACT: copy+accum reduces for slices 5,6 (DVE-multiplied) and 7
  Per batch: DVE tensor_reduce folds the partial tails + ACT accum columns
  into [128, 8] per-k sums; host reorders [p,k]->[d] and divides by Z.
"""

import numpy as np
import ml_dtypes

import concourse.bass as bass
import concourse.bacc as bacc
import concourse.mybir as mybir
import concourse.tile as tile
from concourse.bass_utils import run_bass_kernel_spmd

F32 = mybir.dt.float32
F32R = mybir.dt.float32r
BF16 = mybir.dt.bfloat16
AF = mybir.ActivationFunctionType
ALU = mybir.AluOpType
AX = mybir.AxisListType

N_CORES = 8
B, S, D, F = 32, 4096, 1024, 128
BP = B // N_CORES          # batches per core
KD = D // 128              # d-slices (partition groups)
SC = 1024                  # s-chunk
NCH = S // SC              # chunks per batch
NP = BP * (NCH // 2)       # record-pairs per core (pair = 2 chunks, 1 batch)

NKF = 4                    # k-slices reduced by the DVE fold tree
TW = 64                    # fold-tail width


def build_nc(bp=BP, s=S, d=D, f=F):
    nc = bacc.Bacc("TRN2", target_bir_lowering=False, debug=False)

    xt_dram = nc.dram_tensor("xt", [NP, 128, 2 * KD * SC], BF16,
                             kind="ExternalInput").ap()
    wet_dram = nc.dram_tensor("wet", [128, KD * f], BF16, kind="ExternalInput").ap()
    vrep_dram = nc.dram_tensor("vrep", [f, 128], BF16, kind="ExternalInput").ap()
    vbb_dram = nc.dram_tensor("vbb", [128, 1], F32, kind="ExternalInput").ap()
    web_dram = nc.dram_tensor("web", [f, 1], F32, kind="ExternalInput").ap()
    wdt_dram = nc.dram_tensor("wdt", [128, KD * f], F32R, kind="ExternalInput").ap()
    dht_dram = nc.dram_tensor("dht", [128, KD * bp], F32R, kind="ExternalInput").ap()
    out_dram = nc.dram_tensor("out", [bp, 128, KD], F32, kind="ExternalOutput").ap()
    z_dram = nc.dram_tensor("z", [1, bp], F32, kind="ExternalOutput").ap()

    with tile.TileContext(nc) as tc:
        with (
            tc.tile_pool(name="const", bufs=1) as const,
            tc.tile_pool(name="xpool", bufs=3) as xpool,
            tc.tile_pool(name="mpool", bufs=2) as mpool,
            tc.tile_pool(name="hpool", bufs=3) as hpool,
            tc.tile_pool(name="apool", bufs=2) as apool,
            tc.tile_pool(name="ppool", bufs=2) as ppool,
            tc.tile_pool(name="ptpool", bufs=2) as ptpool,
            tc.tile_pool(name="opool", bufs=2) as opool,
            tc.tile_pool(name="ps_e", bufs=2, space="PSUM") as ps_e,
            tc.tile_pool(name="ps_sc", bufs=1, space="PSUM") as ps_sc,
        ):
            def load_x(j):
                xt = xpool.tile([128, 2, KD, SC], BF16, tag="x", name=f"x_{j}")
                src = xt_dram[j].rearrange("p (r k s) -> p r k s", r=2, k=KD)
                # two half-loads so e_pair(j, r0) can start a half-DMA early
                nc.sync.dma_start(xt[:, 0], src[:, 0])
                nc.sync.dma_start(xt[:, 1], src[:, 1])
                return xt

            # ---- constants (loaded FIRST: the first matmul needs wet) ----
            wet_sb = const.tile([128, KD, f], BF16)
            nc.sync.dma_start(wet_sb, wet_dram.rearrange("p (k f) -> p k f", k=KD))
            vrep_sb = const.tile([f, 128], BF16)
            nc.sync.dma_start(vrep_sb, vrep_dram)
            vbb_sb = const.tile([128, 1], F32)
            nc.sync.dma_start(vbb_sb, vbb_dram)
            web_sb = const.tile([f, 1], F32)
            nc.sync.dma_start(web_sb, web_dram)
            wdt_sb = const.tile([128, KD, f], F32R)
            nc.sync.dma_start(wdt_sb, wdt_dram.rearrange("p (k f) -> p k f", k=KD))
            dht_sb = const.tile([128, KD, bp], F32R)
            nc.sync.dma_start(dht_sb, dht_dram.rearrange("p (k b) -> p k b", k=KD))
            dvec_sb = const.tile([f, bp], F32)
            zcols_sb = const.tile([128, 2 * bp], F32)
            zvals_sb = const.tile([1, bp], F32)

            xts = {j: load_x(j) for j in range(min(3, NP))}

            state = {}
            pts = {}
            ptas = {}

            def get_pt(b):
                if b not in pts:
                    pts[b] = ptpool.tile([128, NKF, NCH, TW], BF16, tag="pt",
                                         name=f"pt_{b}")
                    ptas[b] = ptpool.tile([128, KD - NKF, 2], F32, tag="pta",
                                          name=f"pta_{b}")
                return pts[b], ptas[b]

            def e_pair(j):
                xt = xts[j]
                e0 = ps_e.tile([f, SC], F32, tag="e", name=f"e_{j}_0")
                e1 = ps_e.tile([f, SC], F32, tag="e", name=f"e_{j}_1")
                for r, e_ps in ((0, e0), (1, e1)):
                    for k in range(KD):
                        for h2 in range(2):
                            sl = slice(h2 * 512, (h2 + 1) * 512)
                            nc.tensor.matmul(
                                e_ps[:, sl], wet_sb[:, k, :], xt[:, r, k, sl],
                                start=(k == 0), stop=(k == KD - 1))
                state[j] = {"xt": xt, "e": (e0, e1)}

            def tanh_pair(j):
                b = j // 2
                e0, e1 = state[j].pop("e")
                hs = []
                for r, e_ps in ((0, e0), (1, e1)):
                    h_sb = hpool.tile([f, SC], BF16, tag="h", name=f"h_{j}_{r}")
                    nc.scalar.activation(h_sb, e_ps, AF.Tanh,
                                         bias=dvec_sb[:, b:b + 1])
                    hs.append(h_sb)
                state[j]["h"] = hs

            def scores_pair(j):
                hs = state[j].pop("h")
                sc2 = ps_sc.tile([128, 2, SC], F32, tag="sc", name=f"sc_{j}")
                for r in range(2):
                    for h2 in range(2):
                        sl = slice(h2 * 512, (h2 + 1) * 512)
                        nc.tensor.matmul(sc2[:, r, sl], vrep_sb, hs[r][:, sl],
                                         start=True, stop=True)
                state[j]["sc2"] = sc2

            def expa(j):
                sc2 = state[j].pop("sc2")
                a2 = apool.tile([128, 2, SC], BF16, tag="a", name=f"a_{j}")
                nc.scalar.activation(a2.rearrange("p r s -> p (r s)"),
                                     sc2.rearrange("p r s -> p (r s)"),
                                     AF.Exp, bias=vbb_sb,
                                     accum_out=zcols_sb[:, j:j + 1])
                state[j]["a2"] = a2

            def wsum(j):
                b, half = j // 2, j % 2
                st = state[j]
                xt, a2 = st.pop("xt"), st.pop("a2")
                xts.pop(j)
                pt, _ = get_pt(b)
                # DVE: one broadcast mult for all 8 k-slices of both chunks
                m2 = mpool.tile([128, 2, KD, SC], BF16, tag="m", name=f"m_{j}")
                nc.vector.tensor_tensor(
                    m2, xt[:, :, :, :],
                    a2.unsqueeze(2).broadcast_to([128, 2, KD, SC]),
                    op=ALU.mult)
                # DVE: in-place fold tree on slices 0..NKF-1
                h = SC // 2
                while h >= 2 * TW:
                    nc.vector.tensor_tensor(
                        m2[:, :, 0:NKF, 0:h], m2[:, :, 0:NKF, 0:h],
                        m2[:, :, 0:NKF, h:2 * h], op=ALU.add)
                    h //= 2
                # last fold drops tails into the per-batch partial tile
                nc.vector.tensor_tensor(
                    pt[:, :, 2 * half:2 * half + 2, :],
                    m2[:, :, 0:NKF, 0:TW].rearrange("p r k s -> p k r s"),
                    m2[:, :, 0:NKF, TW:2 * TW].rearrange("p r k s -> p k r s"),
                    op=ALU.add)
                st["m2"] = m2

            def act_reduces(j):
                b, half = j // 2, j % 2
                m2 = state.pop(j)["m2"]
                _, pta = get_pt(b)
                # ACT: pair-wide copy+accum reduces for k=NKF..7 (both chunks
                # of one batch summed together; fin() sums chunks anyway)
                da = ppool.tile([128, 2, SC], BF16, tag="da", name=f"da_{j}")
                for ki in range(NKF, KD):
                    nc.scalar.activation(
                        da, m2[:, :, ki, :], AF.Copy,
                        accum_out=pta[:, ki - NKF, half:half + 1])

            def fin(b):
                pt, pta = pts.pop(b), ptas.pop(b)
                acc = opool.tile([128, KD], F32, tag="acc", name=f"acc_{b}")
                nc.vector.tensor_reduce(
                    acc[:, 0:NKF].unsqueeze(2),
                    pt.rearrange("p k c s -> p k (c s)"), axis=AX.X, op=ALU.add)
                nc.vector.tensor_reduce(
                    acc[:, NKF:KD].unsqueeze(2), pta, axis=AX.X, op=ALU.add)
                nc.vector.tensor_reduce(
                    zvals_sb[0:1, b:b + 1], zcols_sb[0:1, 2 * b:2 * b + 2],
                    axis=AX.X, op=ALU.add)
                nc.gpsimd.dma_start(out_dram[b], acc)

            # ---- d-vector (dv = Wd @ d_hiddens^T + We_b), PSUM via sc pool.
            # Repeated to keep the PE busy while x_0 streams in, so the HAM
            # clock gate opens before the first e-matmul.
            dv_ps = ps_sc.tile([f, bp], F32, tag="sc", name="dv_ps")
            for _rep in range(6):
                for k in range(KD):
                    nc.tensor.matmul(dv_ps, wdt_sb[:, k, :], dht_sb[:, k, :],
                                     start=(k == 0), stop=(k == KD - 1))
            nc.vector.tensor_scalar_add(dvec_sb, dv_ps, web_sb)

            # ---- software-pipelined issue over pairs ----
            for i in range(NP + 4):
                if i + 3 < NP:
                    xts[i + 3] = load_x(i + 3)
                if 0 <= i - 2 < NP:
                    expa(i - 2)
                if 0 <= i - 1 < NP:
                    tanh_pair(i - 1)
                    scores_pair(i - 1)
                if i < NP:
                    e_pair(i)
                if 0 <= i - 2 < NP:
                    wsum(i - 2)
                if 0 <= i - 3 < NP:
                    act_reduces(i - 3)
                    if (i - 3) % 2 == 1:
                        fin((i - 3) // 2)
            nc.gpsimd.dma_start(z_dram, zvals_sb)

    nc.finalize()
    return nc


_NC_CACHE = {}


def _get_nc(key, **kw):
    if key not in _NC_CACHE:
        _NC_CACHE[key] = build_nc(**kw)
    return _NC_CACHE[key]


def make_in_maps(e_hiddens, d_hiddens, We_w, We_b, Wd_w, v_w, v_b, n_cores=N_CORES):
    bp = e_hiddens.shape[0] // n_cores
    bf16 = ml_dtypes.bfloat16

    def arrange(m):  # [D, x] -> [128, KD*x], partition-major tiles
        dd, xx = m.shape
        return np.ascontiguousarray(
            m.reshape(dd // 128, 128, xx).transpose(1, 0, 2).reshape(128, -1))

    wet = arrange(np.ascontiguousarray(We_w.T)).astype(bf16)    # [128, KD*F]
    vrep = np.ascontiguousarray(
        np.repeat(v_w[0][:, None], 128, axis=1)).astype(bf16)   # [F, 128]
    vbb = np.full((128, 1), np.float32(v_b[0]), np.float32)
    web = np.ascontiguousarray(We_b[:, None]).astype(np.float32)
    wdt = arrange(np.ascontiguousarray(Wd_w.T))                 # [128, KD*F]
    maps = []
    for i in range(n_cores):
        xc = e_hiddens[i * bp:(i + 1) * bp]                     # [bp, S, D]
        # xt[pair, p, r*KD*SC + k*SC + s'] = x[b, (2*half+r)*SC+s', k*128+p]
        xt = np.ascontiguousarray(
            xc.reshape(bp, 2, 2, SC, KD, 128).transpose(0, 1, 5, 2, 4, 3)
        ).astype(bf16).reshape(bp * 2, 128, 2 * KD * SC)
        maps.append({
            "xt": xt,
            "wet": wet,
            "vrep": vrep,
            "vbb": vbb,
            "web": web,
            "wdt": wdt,
            "dht": arrange(np.ascontiguousarray(d_hiddens[i * bp:(i + 1) * bp].T)),
        })
    return maps


def kernel(e_hiddens, d_hiddens, length_mask, We_w, We_b, Wd_w, v_w, v_b,
           _trace=False):
    """Full inputs in, full output out.  length_mask is all-ones (the
    reference adds (1-mask)*1e-32, numerically a no-op)."""
    e_hiddens = np.asarray(e_hiddens, np.float32)
    d_hiddens = np.asarray(d_hiddens, np.float32)
    We_w = np.asarray(We_w, np.float32)
    We_b = np.asarray(We_b, np.float32)
    Wd_w = np.asarray(Wd_w, np.float32)
    v_w = np.asarray(v_w, np.float32)
    v_b = np.asarray(v_b, np.float32)

    nc = _get_nc("full")
    in_maps = make_in_maps(e_hiddens, d_hiddens, We_w, We_b, Wd_w, v_w, v_b)
    res = run_bass_kernel_spmd(nc, in_maps, list(range(N_CORES)), trace=_trace)
    outs = []
    for m in res.results:
        o = m["out"].transpose(0, 2, 1).reshape(BP, D)  # [bp,p,k] -> [bp,d]
        outs.append(o.astype(np.float32) / m["z"].reshape(-1, 1))
    out = np.concatenate(outs, axis=0)
    if _trace:
        kernel.last_results = res
    return out


# revision 23
# speedup vs baseline: 1.0511x; 1.0511x over previous
"""Trainium2 Bass kernel for nn_AttnLayer (additive-attention pooling layer).

Reference computation (per batch b):
    e = e_hiddens @ We_w.T + We_b            # [S, F]
    d = Wd_w @ d_hiddens[b]                  # [F]
    h = tanh(d + e)                          # [S, F]
    s = h @ v_w[0] + v_b                     # [S]
    a = softmax(s)                           # [S]
    out[b] = a @ e_hiddens[b]                # [D]

Strategy (8 cores, data-parallel over batch B=32 -> 4 per core):
  x pre-transposed ON HOST to [d-partition, s-free] bf16, packed in PAIRS of
  1024-long s-chunks (one DMA + one exp + one big DVE mult per pair).
  Per pair (2 chunks of the same batch):
    PE : e^T[f,s] = sum_k wet^T @ xt  (bf16, 8 LDW + 16 MM per chunk);
         scores replicated to 128 partitions via v-replicated stationary
    ACT: h = tanh(e^T + d_b)  ;  a2 = exp(sc2 + v_b) over BOTH chunks in one
         op (accum_out -> pair partial of softmax Z)
    weighted sum x*a, split by engine rates:
      DVE: ONE broadcast tensor_tensor (2x bf16 mode) multiplies k-slices
           0..6 of both chunks; slices 0..4 are then reduced by an IN-PLACE
           bf16 halving fold tree (every fold also 2x) whose 32-wide tails
           drop into a per-batch partial tile
      Pool: k=7 mults (kept small: Pool shares an SBUF port with DVE)
      ACT: copy+accum reduces for slices 5,6 (DVE-multiplied) and 7
  Per batch: DVE tensor_reduce folds the partial tails + ACT accum columns
  into [128, 8] per-k sums; host reorders [p,k]->[d] and divides by Z.
"""

import numpy as np
import ml_dtypes

import concourse.bass as bass
import concourse.bacc as bacc
import concourse.mybir as mybir
import concourse.tile as tile
from concourse.bass_utils import run_bass_kernel_spmd

F32 = mybir.dt.float32
F32R = mybir.dt.float32r
BF16 = mybir.dt.bfloat16
AF = mybir.ActivationFunctionType
ALU = mybir.AluOpType
AX = mybir.AxisListType

N_CORES = 8
B, S, D, F = 32, 4096, 1024, 128
BP = B // N_CORES          # batches per core
KD = D // 128              # d-slices (partition groups)
SC = 1024                  # s-chunk
NCH = S // SC              # chunks per batch
NP = BP * (NCH // 2)       # record-pairs per core (pair = 2 chunks, 1 batch)

NKF = 4                    # k-slices reduced by the DVE fold tree
TW = 64                    # fold-tail width


def build_nc(bp=BP, s=S, d=D, f=F):
    nc = bacc.Bacc("TRN2", target_bir_lowering=False, debug=False)

    xt_dram = nc.dram_tensor("xt", [NP, 128, 2 * KD * SC], BF16,
                             kind="ExternalInput").ap()
    wet_dram = nc.dram_tensor("wet", [128, KD * f], BF16, kind="ExternalInput").ap()
    vrep_dram = nc.dram_tensor("vrep", [f, 128], BF16, kind="ExternalInput").ap()
    vbb_dram = nc.dram_tensor("vbb", [128, 1], F32, kind="ExternalInput").ap()
    dvec_dram = nc.dram_tensor("dvec", [f, bp], F32, kind="ExternalInput").ap()
    out_dram = nc.dram_tensor("out", [bp, 128, KD], F32, kind="ExternalOutput").ap()
    z_dram = nc.dram_tensor("z", [1, bp], F32, kind="ExternalOutput").ap()

    with tile.TileContext(nc) as tc:
        with (
            tc.tile_pool(name="const", bufs=1) as const,
            tc.tile_pool(name="xpool", bufs=3) as xpool,
            tc.tile_pool(name="mpool", bufs=2) as mpool,
            tc.tile_pool(name="hpool", bufs=3) as hpool,
            tc.tile_pool(name="apool", bufs=2) as apool,
            tc.tile_pool(name="ppool", bufs=2) as ppool,
            tc.tile_pool(name="ptpool", bufs=2) as ptpool,
            tc.tile_pool(name="opool", bufs=2) as opool,
            tc.tile_pool(name="ps_e", bufs=2, space="PSUM") as ps_e,
            tc.tile_pool(name="ps_sc", bufs=1, space="PSUM") as ps_sc,
        ):
            def load_x(j):
                xt = xpool.tile([128, 2, KD, SC], BF16, tag="x", name=f"x_{j}")
                src = xt_dram[j].rearrange("p (r k s) -> p r k s", r=2, k=KD)
                # two half-loads so e_pair(j, r0) can start a half-DMA early
                nc.sync.dma_start(xt[:, 0], src[:, 0])
                nc.sync.dma_start(xt[:, 1], src[:, 1])
                return xt

            # ---- constants (loaded FIRST: the first matmul needs wet) ----
            wet_sb = const.tile([128, KD, f], BF16)
            nc.sync.dma_start(wet_sb, wet_dram.rearrange("p (k f) -> p k f", k=KD))
            vrep_sb = const.tile([f, 128], BF16)
            nc.sync.dma_start(vrep_sb, vrep_dram)
            vbb_sb = const.tile([128, 1], F32)
            nc.sync.dma_start(vbb_sb, vbb_dram)
            dvec_sb = const.tile([f, bp], F32)
            nc.sync.dma_start(dvec_sb, dvec_dram)
            zcols_sb = const.tile([128, 2 * bp], F32)
            zvals_sb = const.tile([1, bp], F32)

            xts = {j: load_x(j) for j in range(min(3, NP))}

            state = {}
            pts = {}
            ptas = {}

            def get_pt(b):
                if b not in pts:
                    pts[b] = ptpool.tile([128, NKF, NCH, TW], BF16, tag="pt",
                                         name=f"pt_{b}")
                    ptas[b] = ptpool.tile([128, KD - NKF, 2], F32, tag="pta",
                                          name=f"pta_{b}")
                return pts[b], ptas[b]

            def e_pair(j):
                xt = xts[j]
                e0 = ps_e.tile([f, SC], F32, tag="e", name=f"e_{j}_0")
                e1 = ps_e.tile([f, SC], F32, tag="e", name=f"e_{j}_1")
                for r, e_ps in ((0, e0), (1, e1)):
                    for k in range(KD):
                        for h2 in range(2):
                            sl = slice(h2 * 512, (h2 + 1) * 512)
                            nc.tensor.matmul(
                                e_ps[:, sl], wet_sb[:, k, :], xt[:, r, k, sl],
                                start=(k == 0), stop=(k == KD - 1))
                state[j] = {"xt": xt, "e": (e0, e1)}

            def tanh_pair(j):
                b = j // 2
                e0, e1 = state[j].pop("e")
                hs = []
                for r, e_ps in ((0, e0), (1, e1)):
                    h_sb = hpool.tile([f, SC], BF16, tag="h", name=f"h_{j}_{r}")
                    nc.scalar.activation(h_sb, e_ps, AF.Tanh,
                                         bias=dvec_sb[:, b:b + 1])
                    hs.append(h_sb)
                state[j]["h"] = hs

            def scores_pair(j):
                hs = state[j].pop("h")
                sc2 = ps_sc.tile([128, 2, SC], F32, tag="sc", name=f"sc_{j}")
                for r in range(2):
                    for h2 in range(2):
                        sl = slice(h2 * 512, (h2 + 1) * 512)
                        nc.tensor.matmul(sc2[:, r, sl], vrep_sb, hs[r][:, sl],
                                         start=True, stop=True)
                state[j]["sc2"] = sc2

            def expa(j):
                sc2 = state[j].pop("sc2")
                a2 = apool.tile([128, 2, SC], BF16, tag="a", name=f"a_{j}")
                nc.scalar.activation(a2.rearrange("p r s -> p (r s)"),
                                     sc2.rearrange("p r s -> p (r s)"),
                                     AF.Exp, bias=vbb_sb,
                                     accum_out=zcols_sb[:, j:j + 1])
                state[j]["a2"] = a2

            def wsum(j):
                b, half = j // 2, j % 2
                st = state[j]
                xt, a2 = st.pop("xt"), st.pop("a2")
                xts.pop(j)
                pt, _ = get_pt(b)
                # DVE: one broadcast mult for all 8 k-slices of both chunks
                m2 = mpool.tile([128, 2, KD, SC], BF16, tag="m", name=f"m_{j}")
                nc.vector.tensor_tensor(
                    m2, xt[:, :, :, :],
                    a2.unsqueeze(2).broadcast_to([128, 2, KD, SC]),
                    op=ALU.mult)
                # DVE: in-place fold tree on slices 0..NKF-1
                h = SC // 2
                while h >= 2 * TW:
                    nc.vector.tensor_tensor(
                        m2[:, :, 0:NKF, 0:h], m2[:, :, 0:NKF, 0:h],
                        m2[:, :, 0:NKF, h:2 * h], op=ALU.add)
                    h //= 2
                # last fold drops tails into the per-batch partial tile
                nc.vector.tensor_tensor(
                    pt[:, :, 2 * half:2 * half + 2, :],
                    m2[:, :, 0:NKF, 0:TW].rearrange("p r k s -> p k r s"),
                    m2[:, :, 0:NKF, TW:2 * TW].rearrange("p r k s -> p k r s"),
                    op=ALU.add)
                st["m2"] = m2

            def act_reduces(j):
                b, half = j // 2, j % 2
                m2 = state.pop(j)["m2"]
                _, pta = get_pt(b)
                # ACT: pair-wide copy+accum reduces for k=NKF..7 (both chunks
                # of one batch summed together; fin() sums chunks anyway)
                da = ppool.tile([128, 2, SC], BF16, tag="da", name=f"da_{j}")
                for ki in range(NKF, KD):
                    nc.scalar.activation(
                        da, m2[:, :, ki, :], AF.Copy,
                        accum_out=pta[:, ki - NKF, half:half + 1])

            def fin(b):
                pt, pta = pts.pop(b), ptas.pop(b)
                acc = opool.tile([128, KD], F32, tag="acc", name=f"acc_{b}")
                nc.vector.tensor_reduce(
                    acc[:, 0:NKF].unsqueeze(2),
                    pt.rearrange("p k c s -> p k (c s)"), axis=AX.X, op=ALU.add)
                nc.vector.tensor_reduce(
                    acc[:, NKF:KD].unsqueeze(2), pta, axis=AX.X, op=ALU.add)
                nc.vector.tensor_reduce(
                    zvals_sb[0:1, b:b + 1], zcols_sb[0:1, 2 * b:2 * b + 2],
                    axis=AX.X, op=ALU.add)
                nc.gpsimd.dma_start(out_dram[b], acc)

            # ---- PE warmup: cheap bf16 dummy matmuls while x_0 streams in,
            # so the HAM clock gate opens before the first e-matmul.
            warm_ps = ps_sc.tile([f, 128], F32, tag="sc", name="warm_ps")
            for w in range(48):
                nc.tensor.matmul(warm_ps, wet_sb[:, 0, :], wet_sb[:, w % KD, :],
                                 start=(w == 0), stop=(w == 47))

            # ---- software-pipelined issue over pairs ----
            for i in range(NP + 4):
                if i + 3 < NP:
                    xts[i + 3] = load_x(i + 3)
                if 0 <= i - 2 < NP:
                    expa(i - 2)
                if 0 <= i - 1 < NP:
                    tanh_pair(i - 1)
                    scores_pair(i - 1)
                if i < NP:
                    e_pair(i)
                if 0 <= i - 2 < NP:
                    wsum(i - 2)
                if 0 <= i - 3 < NP:
                    act_reduces(i - 3)
                    if (i - 3) % 2 == 1:
                        fin((i - 3) // 2)
            nc.gpsimd.dma_start(z_dram, zvals_sb)

    nc.finalize()
    return nc


_NC_CACHE = {}


def _get_nc(key, **kw):
    if key not in _NC_CACHE:
        _NC_CACHE[key] = build_nc(**kw)
    return _NC_CACHE[key]


def make_in_maps(e_hiddens, d_hiddens, We_w, We_b, Wd_w, v_w, v_b, n_cores=N_CORES):
    bp = e_hiddens.shape[0] // n_cores
    bf16 = ml_dtypes.bfloat16

    def arrange(m):  # [D, x] -> [128, KD*x], partition-major tiles
        dd, xx = m.shape
        return np.ascontiguousarray(
            m.reshape(dd // 128, 128, xx).transpose(1, 0, 2).reshape(128, -1))

    wet = arrange(np.ascontiguousarray(We_w.T)).astype(bf16)    # [128, KD*F]
    vrep = np.ascontiguousarray(
        np.repeat(v_w[0][:, None], 128, axis=1)).astype(bf16)   # [F, 128]
    vbb = np.full((128, 1), np.float32(v_b[0]), np.float32)
    maps = []
    for i in range(n_cores):
        xc = e_hiddens[i * bp:(i + 1) * bp]                     # [bp, S, D]
        # xt[pair, p, r*KD*SC + k*SC + s'] = x[b, (2*half+r)*SC+s', k*128+p]
        xt = np.ascontiguousarray(
            xc.reshape(bp, 2, 2, SC, KD, 128).transpose(0, 1, 5, 2, 4, 3)
        ).astype(bf16).reshape(bp * 2, 128, 2 * KD * SC)
        # dvec[f, b] = Wd @ d_hiddens[b] + We_b (the tanh bias), tiny on host
        dvec = (d_hiddens[i * bp:(i + 1) * bp] @ Wd_w.T).T + We_b[:, None]
        maps.append({
            "xt": xt,
            "wet": wet,
            "vrep": vrep,
            "vbb": vbb,
            "dvec": np.ascontiguousarray(dvec, np.float32),
        })
    return maps


def kernel(e_hiddens, d_hiddens, length_mask, We_w, We_b, Wd_w, v_w, v_b,
           _trace=False):
    """Full inputs in, full output out.  length_mask is all-ones (the
    reference adds (1-mask)*1e-32, numerically a no-op)."""
    e_hiddens = np.asarray(e_hiddens, np.float32)
    d_hiddens = np.asarray(d_hiddens, np.float32)
    We_w = np.asarray(We_w, np.float32)
    We_b = np.asarray(We_b, np.float32)
    Wd_w = np.asarray(Wd_w, np.float32)
    v_w = np.asarray(v_w, np.float32)
    v_b = np.asarray(v_b, np.float32)

    nc = _get_nc("full")
    in_maps = make_in_maps(e_hiddens, d_hiddens, We_w, We_b, Wd_w, v_w, v_b)
    res = run_bass_kernel_spmd(nc, in_maps, list(range(N_CORES)), trace=_trace)
    outs = []
    for m in res.results:
        o = m["out"].transpose(0, 2, 1).reshape(BP, D)  # [bp,p,k] -> [bp,d]
        outs.append(o.astype(np.float32) / m["z"].reshape(-1, 1))
    out = np.concatenate(outs, axis=0)
    if _trace:
        kernel.last_results = res
    return out
